# revision 10
# baseline (speedup 1.0000x reference)
"""Trainium2 Bass kernel for the polar dynamic-conv module.

Contract: kernel(**inputs) takes the FULL unsharded inputs (as produced by
setup_inputs) and returns the full output (out, r_center) tuple, running the
heavy compute on 8 NeuronCores (data-parallel over batch x width-half).

All matmuls run in bf16 on the PE. Precision for the range-image MLP input is
recovered by splitting the G matrix (and the folded MLP1 weights) into
hi/lo bf16 components (K=4 -> 10 rows; matmul cost is N-bound so this is
free). h/m intermediates are bf16; accumulation is fp32 in PSUM.
"""
import os
import sys

for _p in ('/opt/trn_rl_repo', '/root/.axon_site/_ro/trn_rl_repo'):
    if os.path.isdir(_p) and _p not in sys.path:
        sys.path.insert(0, _p)
        break

import numpy as np
import ml_dtypes
import concourse.bass as bass  # noqa: F401
import concourse.tile as tile
from concourse import bacc, mybir
from concourse.bass_utils import run_bass_kernel_spmd

dt = mybir.dt
AF = mybir.ActivationFunctionType
ALU = mybir.AluOpType

# Problem geometry (hardcoded per the harness contract)
KS, ST, PD = 4, 2, 1
AZI = 2.0 * np.pi / 1024.0
INC = 0.0073
C_IN, C_OUT = 64, 128
B, H, W = 4, 64, 1024
HO, WO = 32, 512
WOC = 256
NCORE = 8
NCHUNK = 16
HPAD, WPADC = 66, 514
KG = 10   # G rows after hi/lo split

_CACHE = {}


def _bf16(a):
    return np.asarray(a, np.float32).astype(ml_dtypes.bfloat16)


def _hi_lo(a):
    """Split fp32 into hi = bf16(a), lo = bf16(a - hi)."""
    a = np.asarray(a, np.float32)
    hi = a.astype(ml_dtypes.bfloat16)
    lo = (a - hi.astype(np.float32)).astype(ml_dtypes.bfloat16)
    return hi, lo


def _build_nc(n_chunks: int = NCHUNK):
    nc = bacc.Bacc("TRN2", target_bir_lowering=False, debug=False)
    xd_d = nc.dram_tensor("XD", [128, HPAD, WPADC], dt.bfloat16,
                          kind="ExternalInput").ap()
    # G/L1 laid out for row-group tiling: pair p lives at partition strip
    # 32*(p%4), free index s = p//4
    g_d = nc.dram_tensor("G", [NCHUNK, 128, 2, 512], dt.bfloat16,
                         kind="ExternalInput").ap()
    l1_d = nc.dram_tensor("L1", [128, 2, 128], dt.bfloat16,
                          kind="ExternalInput").ap()
    l2_d = nc.dram_tensor("L2", [128, 128], dt.bfloat16,
                          kind="ExternalInput").ap()
    lwc_d = nc.dram_tensor("LWc", [128, 8, 128], dt.bfloat16,
                           kind="ExternalInput").ap()
    b2_d = nc.dram_tensor("B2", [128, 1], dt.float32, kind="ExternalInput").ap()
    bc_d = nc.dram_tensor("BC", [128, 1], dt.float32, kind="ExternalInput").ap()
    out_d = nc.dram_tensor("OUT", [128, HO, WOC], dt.float32,
                           kind="ExternalOutput").ap()

    with tile.TileContext(nc) as tc:
        with tc.tile_pool(name="const", bufs=1) as cpool, \
             tc.tile_pool(name="xin", bufs=4) as xpool, \
             tc.tile_pool(name="gin", bufs=4) as gpool, \
             tc.tile_pool(name="hbuf", bufs=4) as hpool, \
             tc.tile_pool(name="mbuf", bufs=8) as mpool, \
             tc.tile_pool(name="obuf", bufs=2) as opool, \
             tc.tile_pool(name="ps_h", bufs=2, space="PSUM") as ps_h, \
             tc.tile_pool(name="ps_w", bufs=2, space="PSUM") as ps_w, \
             tc.tile_pool(name="ps_o", bufs=2, space="PSUM") as ps_o:

            l1 = cpool.tile([128, 2, 128], dt.bfloat16, tag="l1")
            l2 = cpool.tile([128, 128], dt.bfloat16, tag="l2")
            lwc = cpool.tile([128, 8, 128], dt.bfloat16, tag="lwc")
            b2t = cpool.tile([128, 1], dt.float32, tag="b2")
            bct = cpool.tile([128, 1], dt.float32, tag="bc")
            nc.sync.dma_start(l1[:], l1_d[:])
            nc.sync.dma_start(l2[:], l2_d[:])
            nc.sync.dma_start(lwc[:], lwc_d[:])
            nc.sync.dma_start(b2t[:], b2_d[:])
            nc.sync.dma_start(bct[:], bc_d[:])

            for t in range(n_chunks):
                xt = xpool.tile([128, 6, WPADC], dt.bfloat16, tag="xt")
                nc.sync.dma_start(xt[:], xd_d[:, 4 * t:4 * t + 6, :])
                gt = gpool.tile([128, 2, 512], dt.bfloat16, tag="gt")
                nc.sync.dma_start(gt[:], g_d[t])

                op = ps_o.tile([128, 512], dt.float32, tag="op")
                hps = {}
                for s in range(2):
                    # 4 concurrent K=KG matmuls in disjoint 32-row strips
                    for pd in range(2):
                        hp = ps_h.tile([128, 1024], dt.float32, tag="hp")
                        hps[(s, pd)] = hp
                        for k in range(2):
                            q = 2 * pd + k
                            nc.tensor.matmul(
                                hp[:, 512 * k:512 * (k + 1)],
                                l1[32 * q:32 * q + KG, s, :],
                                gt[32 * q:32 * q + KG, s, :],
                                start=True, stop=True,
                                tile_position=(32 * q, 0))
                for s in range(2):
                    for pd in range(2):
                        hp = hps[(s, pd)]
                        h2 = hpool.tile([128, 1024], dt.bfloat16, tag="h2")
                        nc.scalar.activation(h2[:], hp[:], AF.Prelu, alpha=0.2)
                        for k in range(2):
                            q = 2 * pd + k
                            p = 4 * s + q
                            i, jp = p // 2, p % 2
                            wp = ps_w.tile([128, 512], dt.float32, tag="wp")
                            nc.tensor.matmul(wp[:], l2[:],
                                             h2[:, 512 * k:512 * (k + 1)],
                                             start=True, stop=True)
                            xu = xt[:, i:i + 3:2, jp:jp + 511:2]
                            m = mpool.tile([128, 2, 256], dt.bfloat16, tag="m")
                            nc.vector.scalar_tensor_tensor(
                                m[:], wp[:].rearrange("p (a b) -> p a b", a=2),
                                b2t[:], xu, ALU.add, ALU.mult)
                            nc.tensor.matmul(op[:], lwc[:, p, :],
                                             m[:].rearrange("p a b -> p (a b)"),
                                             start=(p == 0), stop=(p == 7))
                os = opool.tile([128, 2, 256], dt.float32, tag="os")
                nc.scalar.activation(
                    os[:], op[:].rearrange("p (a b) -> p a b", a=2),
                    AF.Identity, bias=bct[:])
                nc.sync.dma_start(out_d[:, 2 * t:2 * t + 2, :], os[:])

    nc.compile()
    return nc


def _host_prep(x, r, W1, b1, W2, b2, Wc, bc):
    """Build per-core input maps."""
    x = np.asarray(x, np.float32)
    r = np.asarray(r, np.float32)
    W1 = np.asarray(W1, np.float32)
    b1 = np.asarray(b1, np.float32)
    W2 = np.asarray(W2, np.float32)
    b2 = np.asarray(b2, np.float32)
    Wc = np.asarray(Wc, np.float32)
    bc = np.asarray(bc, np.float32)

    d = np.arange(KS, dtype=np.float64) - KS // 2
    cosA, sinA = np.cos(AZI * d), np.sin(AZI * d)   # indexed by i (row offset)
    cosI, sinI = np.cos(INC * d), np.sin(INC * d)   # indexed by j (col offset)

    u = np.zeros((KS, KS, C_IN), np.float64)
    for j in range(KS):
        for i in range(KS):
            v = np.array([cosA[i] * cosI[j], cosA[i] * sinI[j], sinA[i]])
            u[j, i] = W1.astype(np.float64).T @ v
    w1r0 = W1[0, :].astype(np.float64)

    # L1 [128, 2, 128]: pair p at partition strip 32*(p%4), free idx p//4,
    # with hi/lo split of the weight rows.
    # G row layout (KG=10): 0 rpA_hi, 1 rpA_lo, 2 rpA_hi(dup),
    #                       3 rpB_hi, 4 rpB_lo, 5 rpB_hi(dup),
    #                       6 rc_hi,  7 rc_lo,  8 rc_hi(dup), 9 ones
    # matching lhsT rows:   0 uA_hi,  1 uA_hi,  2 uA_lo,
    #                       3 uB_hi,  4 uB_hi,  5 uB_lo,
    #                       6 -w_hi,  7 -w_hi,  8 -w_lo,     9 b1
    L1 = np.zeros((128, 2, 128), ml_dtypes.bfloat16)
    for p in range(8):
        i, jp = p // 2, p % 2
        q, s = p % 4, p // 4
        r0 = 32 * q
        uA_hi, uA_lo = _hi_lo(u[jp, i])
        uB_hi, uB_lo = _hi_lo(u[jp + 2, i])
        w_hi, w_lo = _hi_lo(w1r0)
        L1[r0 + 0, s, 0:64] = uA_hi
        L1[r0 + 1, s, 0:64] = uA_hi
        L1[r0 + 2, s, 0:64] = uA_lo
        L1[r0 + 3, s, 64:128] = uB_hi
        L1[r0 + 4, s, 64:128] = uB_hi
        L1[r0 + 5, s, 64:128] = uB_lo
        for half in (slice(0, 64), slice(64, 128)):
            L1[r0 + 6, s, half] = -w_hi
            L1[r0 + 7, s, half] = -w_hi
            L1[r0 + 8, s, half] = -w_lo
            L1[r0 + 9, s, half] = _bf16(b1)

    L2 = np.zeros((128, 128), ml_dtypes.bfloat16)
    L2[0:64, 0:64] = _bf16(W2)
    L2[64:128, 64:128] = _bf16(W2)

    Wc4 = Wc.reshape(C_OUT, C_IN, KS, KS)  # [o, c, j, i]
    LWc = np.zeros((128, 8, 128), ml_dtypes.bfloat16)
    for p in range(8):
        i, jp = p // 2, p % 2
        LWc[0:64, p, :] = _bf16(Wc4[:, :, jp, i].T)
        LWc[64:128, p, :] = _bf16(Wc4[:, :, jp + 2, i].T)

    B2 = np.concatenate([b2, b2]).reshape(128, 1).astype(np.float32)
    BC = bc.reshape(128, 1).astype(np.float32)

    # Padded x / r
    xw = np.zeros((B, C_IN, H, W + 2), np.float32)
    xw[:, :, :, 1:-1] = x
    rw = np.full((B, 1, H, W + 2), 100.0, np.float32)
    rw[:, :, :, 1:-1] = r

    in_maps = []
    wo2 = 2 * np.arange(WOC)
    for core in range(NCORE):
        b, wh = divmod(core, 2)
        xc = xw[b, :, :, wh * 512: wh * 512 + WPADC]
        xc = np.concatenate([xc[:, -1:, :], xc, xc[:, :1, :]], axis=1)
        rc_ = rw[b, 0, :, wh * 512: wh * 512 + WPADC]
        rc_ = np.concatenate([rc_[-1:], rc_, rc_[:1]], axis=0)

        XD = np.zeros((128, HPAD, WPADC), ml_dtypes.bfloat16)
        XD[0:64] = _bf16(xc)
        XD[64:128, :, 0:WPADC - 2] = _bf16(xc[:, :, 2:])

        G = np.zeros((NCHUNK, 128, 2, 512), ml_dtypes.bfloat16)
        for t in range(NCHUNK):
            for p in range(8):
                i, jp = p // 2, p % 2
                q, s = p % 4, p // 4
                r0 = 32 * q
                for dh in range(2):
                    row = 4 * t + 2 * dh
                    sl = slice(dh * 256, dh * 256 + 256)
                    rpA = rc_[row + i, jp + wo2]
                    rpB = rc_[row + i, jp + 2 + wo2]
                    rcv = rc_[row + 2, 2 + wo2]
                    for base, val in ((0, rpA), (3, rpB), (6, rcv)):
                        hi, lo = _hi_lo(val)
                        G[t, r0 + base + 0, s, sl] = hi
                        G[t, r0 + base + 1, s, sl] = lo
                        G[t, r0 + base + 2, s, sl] = hi
                    G[t, r0 + 9, s, sl] = 1.0

        in_maps.append({"XD": XD, "G": G, "L1": L1, "L2": L2,
                        "LWc": LWc, "B2": B2, "BC": BC})
    return in_maps


def kernel(x, r, W1, b1, W2, b2, Wc, bc):
    if "nc" not in _CACHE:
        _CACHE["nc"] = _build_nc()
    nc = _CACHE["nc"]
    in_maps = _host_prep(x, r, W1, b1, W2, b2, Wc, bc)
    trace = bool(int(os.environ.get("DYNCONV_TRACE", "0")))
    res = run_bass_kernel_spmd(nc, in_maps, list(range(NCORE)), trace=trace)
    if trace:
        _CACHE["exec_time_ns"] = res.exec_time_ns
        _CACHE["mean_exec_time_ns"] = res.mean_exec_time_ns
        _CACHE["results_obj"] = res

    out_full = np.empty((B, C_OUT, HO, WO), np.float32)
    for core in range(NCORE):
        b, wh = divmod(core, 2)
        out_full[b, :, :, wh * WOC:(wh + 1) * WOC] = res.results[core]["OUT"]
    r_center = np.asarray(r, np.float32)[:, :, 1::2, 1::2]
    return out_full, r_center


# revision 11
# speedup vs baseline: 1.1056x; 1.1056x over previous
"""Trainium2 Bass kernel for the polar dynamic-conv module.

Contract: kernel(**inputs) takes the FULL unsharded inputs (as produced by
setup_inputs) and returns the full output (out, r_center) tuple, running the
heavy compute on 8 NeuronCores (data-parallel over batch x width-half).

All matmuls run in bf16 on the PE. Precision for the range-image MLP input is
recovered by splitting the G matrix (and the folded MLP1 weights) into
hi/lo bf16 components (K=4 -> 10 rows; matmul cost is N-bound so this is
free). h/m intermediates are bf16; accumulation is fp32 in PSUM.
"""
import os
import sys

for _p in ('/opt/trn_rl_repo', '/root/.axon_site/_ro/trn_rl_repo'):
    if os.path.isdir(_p) and _p not in sys.path:
        sys.path.insert(0, _p)
        break

import numpy as np
import ml_dtypes
import concourse.bass as bass  # noqa: F401
import concourse.tile as tile
from concourse import bacc, mybir
from concourse.bass_utils import run_bass_kernel_spmd

dt = mybir.dt
AF = mybir.ActivationFunctionType
ALU = mybir.AluOpType

# Problem geometry (hardcoded per the harness contract)
KS, ST, PD = 4, 2, 1
AZI = 2.0 * np.pi / 1024.0
INC = 0.0073
C_IN, C_OUT = 64, 128
B, H, W = 4, 64, 1024
HO, WO = 32, 512
WOC = 256
NCORE = 8
NCHUNK = 16
HPAD, WPADC = 66, 514
KG = 10   # G rows after hi/lo split

_CACHE = {}


def _bf16(a):
    return np.asarray(a, np.float32).astype(ml_dtypes.bfloat16)


def _hi_lo(a):
    """Split fp32 into hi = bf16(a), lo = bf16(a - hi)."""
    a = np.asarray(a, np.float32)
    hi = a.astype(ml_dtypes.bfloat16)
    lo = (a - hi.astype(np.float32)).astype(ml_dtypes.bfloat16)
    return hi, lo


def _build_nc(n_chunks: int = NCHUNK):
    nc = bacc.Bacc("TRN2", target_bir_lowering=False, debug=False)
    xd_d = nc.dram_tensor("XD", [128, HPAD, WPADC], dt.bfloat16,
                          kind="ExternalInput").ap()
    # G/L1 laid out for row-group tiling: pair p lives at partition strip
    # 32*(p%4), free index s = p//4
    g_d = nc.dram_tensor("G", [NCHUNK, 128, 2, 512], dt.bfloat16,
                         kind="ExternalInput").ap()
    l1_d = nc.dram_tensor("L1", [128, 2, 128], dt.bfloat16,
                          kind="ExternalInput").ap()
    l2_d = nc.dram_tensor("L2", [128, 128], dt.bfloat16,
                          kind="ExternalInput").ap()
    lwc_d = nc.dram_tensor("LWc", [128, 8, 128], dt.bfloat16,
                           kind="ExternalInput").ap()
    b2_d = nc.dram_tensor("B2", [128, 1], dt.float32, kind="ExternalInput").ap()
    bc_d = nc.dram_tensor("BC", [128, 1], dt.float32, kind="ExternalInput").ap()
    out_d = nc.dram_tensor("OUT", [128, HO, WOC], dt.float32,
                           kind="ExternalOutput").ap()

    with tile.TileContext(nc) as tc:
        with tc.tile_pool(name="const", bufs=1) as cpool, \
             tc.tile_pool(name="xin", bufs=3) as xpool, \
             tc.tile_pool(name="gin", bufs=3) as gpool, \
             tc.tile_pool(name="hbuf", bufs=3) as hpool, \
             tc.tile_pool(name="mbuf", bufs=3) as mpool, \
             tc.tile_pool(name="obuf", bufs=2) as opool, \
             tc.tile_pool(name="ps_h", bufs=2, space="PSUM") as ps_h, \
             tc.tile_pool(name="ps_w", bufs=2, space="PSUM") as ps_w, \
             tc.tile_pool(name="ps_o", bufs=2, space="PSUM") as ps_o:

            l1 = cpool.tile([128, 2, 128], dt.bfloat16, tag="l1")
            l2 = cpool.tile([128, 128], dt.bfloat16, tag="l2")
            lwc = cpool.tile([128, 8, 128], dt.bfloat16, tag="lwc")
            b2t = cpool.tile([128, 1], dt.float32, tag="b2")
            bct = cpool.tile([128, 1], dt.float32, tag="bc")
            nc.sync.dma_start(l1[:], l1_d[:])
            nc.sync.dma_start(l2[:], l2_d[:])
            nc.sync.dma_start(lwc[:], lwc_d[:])
            nc.sync.dma_start(b2t[:], b2_d[:])
            nc.sync.dma_start(bct[:], bc_d[:])

            for t in range(n_chunks):
                xt = xpool.tile([128, 6, WPADC], dt.bfloat16, tag="xt")
                nc.sync.dma_start(xt[:], xd_d[:, 4 * t:4 * t + 6, :])
                gt = gpool.tile([128, 2, 512], dt.bfloat16, tag="gt")
                nc.sync.dma_start(gt[:], g_d[t])

                op = ps_o.tile([128, 512], dt.float32, tag="op")
                hps = {}
                for s in range(2):
                    # 4 concurrent K=KG matmuls in disjoint 32-row strips
                    for pd in range(2):
                        hp = ps_h.tile([128, 1024], dt.float32, tag="hp")
                        hps[(s, pd)] = hp
                        for k in range(2):
                            q = 2 * pd + k
                            nc.tensor.matmul(
                                hp[:, 512 * k:512 * (k + 1)],
                                l1[32 * q:32 * q + KG, s, :],
                                gt[32 * q:32 * q + KG, s, :],
                                start=True, stop=True,
                                tile_position=(32 * q, 0))
                for s in range(2):
                    for pd in range(2):
                        hp = hps[(s, pd)]
                        h2 = hpool.tile([128, 1024], dt.bfloat16, tag="h2")
                        nc.scalar.activation(h2[:], hp[:], AF.Prelu, alpha=0.2)
                        for k in range(2):
                            q = 2 * pd + k
                            p = 4 * s + q
                            i, jp = p // 2, p % 2
                            wp = ps_w.tile([128, 512], dt.float32, tag="wp")
                            nc.tensor.matmul(wp[:], l2[:],
                                             h2[:, 512 * k:512 * (k + 1)],
                                             start=True, stop=True)
                            xu = xt[:, i:i + 3:2, jp:jp + 511:2]
                            m = mpool.tile([128, 2, 256], dt.bfloat16, tag="m")
                            nc.vector.scalar_tensor_tensor(
                                m[:], wp[:].rearrange("p (a b) -> p a b", a=2),
                                b2t[:], xu, ALU.add, ALU.mult)
                            nc.tensor.matmul(op[:], lwc[:, p, :],
                                             m[:].rearrange("p a b -> p (a b)"),
                                             start=(p == 0), stop=(p == 7))
                os = opool.tile([128, 2, 256], dt.float32, tag="os")
                nc.scalar.activation(
                    os[:], op[:].rearrange("p (a b) -> p a b", a=2),
                    AF.Identity, bias=bct[:])
                nc.sync.dma_start(out_d[:, 2 * t:2 * t + 2, :], os[:])

    nc.compile()
    return nc


def _host_prep(x, r, W1, b1, W2, b2, Wc, bc):
    """Build per-core input maps."""
    x = np.asarray(x, np.float32)
    r = np.asarray(r, np.float32)
    W1 = np.asarray(W1, np.float32)
    b1 = np.asarray(b1, np.float32)
    W2 = np.asarray(W2, np.float32)
    b2 = np.asarray(b2, np.float32)
    Wc = np.asarray(Wc, np.float32)
    bc = np.asarray(bc, np.float32)

    d = np.arange(KS, dtype=np.float64) - KS // 2
    cosA, sinA = np.cos(AZI * d), np.sin(AZI * d)   # indexed by i (row offset)
    cosI, sinI = np.cos(INC * d), np.sin(INC * d)   # indexed by j (col offset)

    u = np.zeros((KS, KS, C_IN), np.float64)
    for j in range(KS):
        for i in range(KS):
            v = np.array([cosA[i] * cosI[j], cosA[i] * sinI[j], sinA[i]])
            u[j, i] = W1.astype(np.float64).T @ v
    w1r0 = W1[0, :].astype(np.float64)

    # L1 [128, 2, 128]: pair p at partition strip 32*(p%4), free idx p//4,
    # with hi/lo split of the weight rows.
    # G row layout (KG=10): 0 rpA_hi, 1 rpA_lo, 2 rpA_hi(dup),
    #                       3 rpB_hi, 4 rpB_lo, 5 rpB_hi(dup),
    #                       6 rc_hi,  7 rc_lo,  8 rc_hi(dup), 9 ones
    # matching lhsT rows:   0 uA_hi,  1 uA_hi,  2 uA_lo,
    #                       3 uB_hi,  4 uB_hi,  5 uB_lo,
    #                       6 -w_hi,  7 -w_hi,  8 -w_lo,     9 b1
    L1 = np.zeros((128, 2, 128), ml_dtypes.bfloat16)
    for p in range(8):
        i, jp = p // 2, p % 2
        q, s = p % 4, p // 4
        r0 = 32 * q
        uA_hi, uA_lo = _hi_lo(u[jp, i])
        uB_hi, uB_lo = _hi_lo(u[jp + 2, i])
        w_hi, w_lo = _hi_lo(w1r0)
        L1[r0 + 0, s, 0:64] = uA_hi
        L1[r0 + 1, s, 0:64] = uA_hi
        L1[r0 + 2, s, 0:64] = uA_lo
        L1[r0 + 3, s, 64:128] = uB_hi
        L1[r0 + 4, s, 64:128] = uB_hi
        L1[r0 + 5, s, 64:128] = uB_lo
        for half in (slice(0, 64), slice(64, 128)):
            L1[r0 + 6, s, half] = -w_hi
            L1[r0 + 7, s, half] = -w_hi
            L1[r0 + 8, s, half] = -w_lo
            L1[r0 + 9, s, half] = _bf16(b1)

    L2 = np.zeros((128, 128), ml_dtypes.bfloat16)
    L2[0:64, 0:64] = _bf16(W2)
    L2[64:128, 64:128] = _bf16(W2)

    Wc4 = Wc.reshape(C_OUT, C_IN, KS, KS)  # [o, c, j, i]
    LWc = np.zeros((128, 8, 128), ml_dtypes.bfloat16)
    for p in range(8):
        i, jp = p // 2, p % 2
        LWc[0:64, p, :] = _bf16(Wc4[:, :, jp, i].T)
        LWc[64:128, p, :] = _bf16(Wc4[:, :, jp + 2, i].T)

    B2 = np.concatenate([b2, b2]).reshape(128, 1).astype(np.float32)
    BC = bc.reshape(128, 1).astype(np.float32)

    # Padded x / r
    xw = np.zeros((B, C_IN, H, W + 2), np.float32)
    xw[:, :, :, 1:-1] = x
    rw = np.full((B, 1, H, W + 2), 100.0, np.float32)
    rw[:, :, :, 1:-1] = r

    in_maps = []
    wo2 = 2 * np.arange(WOC)
    for core in range(NCORE):
        b, wh = divmod(core, 2)
        xc = xw[b, :, :, wh * 512: wh * 512 + WPADC]
        xc = np.concatenate([xc[:, -1:, :], xc, xc[:, :1, :]], axis=1)
        rc_ = rw[b, 0, :, wh * 512: wh * 512 + WPADC]
        rc_ = np.concatenate([rc_[-1:], rc_, rc_[:1]], axis=0)

        XD = np.zeros((128, HPAD, WPADC), ml_dtypes.bfloat16)
        XD[0:64] = _bf16(xc)
        XD[64:128, :, 0:WPADC - 2] = _bf16(xc[:, :, 2:])

        G = np.zeros((NCHUNK, 128, 2, 512), ml_dtypes.bfloat16)
        for t in range(NCHUNK):
            for p in range(8):
                i, jp = p // 2, p % 2
                q, s = p % 4, p // 4
                r0 = 32 * q
                for dh in range(2):
                    row = 4 * t + 2 * dh
                    sl = slice(dh * 256, dh * 256 + 256)
                    rpA = rc_[row + i, jp + wo2]
                    rpB = rc_[row + i, jp + 2 + wo2]
                    rcv = rc_[row + 2, 2 + wo2]
                    for base, val in ((0, rpA), (3, rpB), (6, rcv)):
                        hi, lo = _hi_lo(val)
                        G[t, r0 + base + 0, s, sl] = hi
                        G[t, r0 + base + 1, s, sl] = lo
                        G[t, r0 + base + 2, s, sl] = hi
                    G[t, r0 + 9, s, sl] = 1.0

        in_maps.append({"XD": XD, "G": G, "L1": L1, "L2": L2,
                        "LWc": LWc, "B2": B2, "BC": BC})
    return in_maps


def kernel(x, r, W1, b1, W2, b2, Wc, bc):
    if "nc" not in _CACHE:
        _CACHE["nc"] = _build_nc()
    nc = _CACHE["nc"]
    in_maps = _host_prep(x, r, W1, b1, W2, b2, Wc, bc)
    trace = bool(int(os.environ.get("DYNCONV_TRACE", "0")))
    res = run_bass_kernel_spmd(nc, in_maps, list(range(NCORE)), trace=trace)
    if trace:
        _CACHE["exec_time_ns"] = res.exec_time_ns
        _CACHE["mean_exec_time_ns"] = res.mean_exec_time_ns
        _CACHE["results_obj"] = res

    out_full = np.empty((B, C_OUT, HO, WO), np.float32)
    for core in range(NCORE):
        b, wh = divmod(core, 2)
        out_full[b, :, :, wh * WOC:(wh + 1) * WOC] = res.results[core]["OUT"]
    r_center = np.asarray(r, np.float32)[:, :, 1::2, 1::2]
    return out_full, r_center
